# revision 23
# baseline (speedup 1.0000x reference)
"""Multi-head attention TRN2 kernel (8 NeuronCores, SPMD).

Problem: B=2, N=2048, D=1024, H=16 heads of dim 64, fp32 in/out, per-(b,h)
key-length masking (valid_len, length 32).

Sharding: batch*heads across 8 cores - core c handles batch b=c//4 and 4
heads ("slots"), rank-aligned by valid_len so the per-slot key-tile trip
counts (uniform across cores, specialized at build time) are minimal.

v4 design (HAM-warm, ACT-paced, minimal serial head):
  - All matmul operands bf16 (host converts; PSUM accumulation stays fp32).
  - Junk matmuls at t=0 warm the HAM clock gate while weights DMA.
  - Serial head is only: V proj k-tiles 0..7, K proj chunk 0, Q proj chunk
    0. The rest of K proj and V proj ride as fillers inside the first
    attention ladder, using the spare "mix" PSUM banks; Q proj of chunk
    q+1 and out-proj of chunk q-1 ride inside later ladders.
  - S^T per slot-pair: two K=64 matmuls emitted back-to-back run
    concurrently in different PE row-groups (tile_position auto-derived).
    Both halves land in one [128,1024] 2-bank PSUM tile; ONE fused Exp
    (scale=1/8) covers both slots -> bf16 pT.
  - Softmax denominator via masked-ones column in V1.
  - Normalize: DVE reciprocal_approx_fast (SBUF-staged; the custom op
    misreads PSUM on HW) + GpSimd partition_broadcast + DVE multiply.
    Slot b normalizes as soon as its accumulator stops (hidden in ladder).
  - PSUM budget (8 banks): sT 2x2 + acc2 2 + mix 2.
"""
import sys
import numpy as np
from contextlib import ExitStack

sys.path.insert(0, "/opt/trn_rl_repo")

import ml_dtypes  # noqa: E402
import concourse.bass as bass  # noqa: E402
from concourse import bacc, mybir  # noqa: E402
import concourse.tile as tile  # noqa: E402
from concourse.bass_utils import run_bass_kernel_spmd  # noqa: E402

F32 = mybir.dt.float32
BF16 = mybir.dt.bfloat16
AF = mybir.ActivationFunctionType
MUL = mybir.AluOpType.mult
BFNP = ml_dtypes.bfloat16

B, N, D, H = 2, 2048, 1024, 16
DH = 64
HPC = 4          # heads (slots) per core
NCORES = 8
QC = 512         # q chunk (matmul free dim)
NKT = N // 128   # 16 k tiles
NDC = D // 128   # 8 contraction chunks
NCH = N // QC    # 4 q chunks

LAST_RESULTS = None  # BassKernelResults of the most recent run (for tooling)
DEBUG_DUMP = False   # add DRAM dumps of intermediates (debugging only)


def _build_program(trips):
    """trips: 4 ints - k-tile count per slot (uniform across cores)."""
    nc = bacc.Bacc("TRN2", target_bir_lowering=False, debug=False,
                   num_devices=NCORES)
    maxtr = max(trips)

    xTq = nc.dram_tensor("xTq", [D, N], BF16, kind="ExternalInput")
    xTk = nc.dram_tensor("xTk", [D, N], BF16, kind="ExternalInput")
    xTv = nc.dram_tensor("xTv", [D, N], BF16, kind="ExternalInput")
    wq = nc.dram_tensor("wq", [128, NDC * 256], BF16, kind="ExternalInput")
    wk = nc.dram_tensor("wk", [128, NDC * 256], BF16, kind="ExternalInput")
    wv = nc.dram_tensor("wv", [128, NDC * 256], BF16, kind="ExternalInput")
    wo = nc.dram_tensor("wo", [256, D], BF16, kind="ExternalInput")
    vmask = nc.dram_tensor("vmask", [128, HPC * NKT], F32, kind="ExternalInput")
    out = nc.dram_tensor("out", [N, D], F32, kind="ExternalOutput")
    dbg = {}
    if DEBUG_DUMP:
        for p in range(2):
            dbg[f"qT{p}"] = nc.dram_tensor(f"d_qT{p}", [128, N], BF16,
                                           kind="ExternalOutput")
            dbg[f"kT{p}"] = nc.dram_tensor(f"d_kT{p}", [128, N], BF16,
                                           kind="ExternalOutput")
            dbg[f"pb{p}"] = nc.dram_tensor(f"d_pb{p}", [128, N], BF16,
                                           kind="ExternalOutput")
        for j in range(HPC):
            dbg[f"v1_{j}"] = nc.dram_tensor(f"d_v1_{j}", [128, 65 * trips[j]],
                                            BF16, kind="ExternalOutput")
        dbg["sT00"] = nc.dram_tensor("d_sT00", [128, 1024], F32,
                                     kind="ExternalOutput")
        dbg["pT00"] = nc.dram_tensor("d_pT00", [128, 1024], BF16,
                                     kind="ExternalOutput")
        dbg["acc_a00"] = nc.dram_tensor("d_acc_a00", [65, QC], F32,
                                        kind="ExternalOutput")
        dbg["acc_b00"] = nc.dram_tensor("d_acc_b00", [65, QC], F32,
                                        kind="ExternalOutput")
        dbg["r00"] = nc.dram_tensor("d_r00", [2, QC], F32,
                                    kind="ExternalOutput")

    with tile.TileContext(nc) as tc:
        with ExitStack() as ctx:
            wpool = ctx.enter_context(tc.tile_pool(name="wpool", bufs=1))
            xvpool = ctx.enter_context(tc.tile_pool(name="xvpool", bufs=16))
            xkpool = ctx.enter_context(tc.tile_pool(name="xkpool", bufs=32))
            qxpool = ctx.enter_context(tc.tile_pool(name="qxpool", bufs=16))
            qkpool = ctx.enter_context(tc.tile_pool(name="qkpool", bufs=1))
            v1pool = ctx.enter_context(tc.tile_pool(name="v1pool", bufs=1))
            ptpool = ctx.enter_context(tc.tile_pool(name="ptpool", bufs=3))
            nrmpool = ctx.enter_context(tc.tile_pool(name="nrmpool", bufs=2))
            pbpool = ctx.enter_context(tc.tile_pool(name="pbpool", bufs=1))
            opool = ctx.enter_context(tc.tile_pool(name="opool", bufs=3))
            ap = ctx.enter_context(tc.tile_pool(name="ap", bufs=1,
                                                space="PSUM"))

            # ---- HAM warm-up: junk matmuls while weights stream in ----
            t_junk = wpool.tile([128, QC], BF16, tag="junk")
            nc.vector.memset(t_junk[:], 0.0)
            for i in range(10):
                wps = ap.tile([128, QC], F32, tag="mix", bufs=2,
                              name=f"warm{i}")
                nc.tensor.matmul(wps[:], t_junk[:, 0:128], t_junk[:],
                                 start=True, stop=True)

            t_wq = wpool.tile([128, NDC * 256], BF16, tag="wq")
            t_wk = wpool.tile([128, NDC * 256], BF16, tag="wk")
            t_wv = wpool.tile([128, NDC * 256], BF16, tag="wv")
            t_vm = wpool.tile([128, HPC * NKT], F32, tag="vm")
            t_wo = [wpool.tile([128, D], BF16, tag=f"wo{p}", name=f"t_wo{p}")
                    for p in range(2)]

            # warm the ACT exp table during the projection head
            t_scr = wpool.tile([1, 1], F32, tag="scr")
            nc.vector.memset(t_scr[:], 0.0)
            t_scr2 = wpool.tile([1, 1], F32, tag="scr2")
            nc.scalar.activation(t_scr2[:], t_scr[:], AF.Exp)

            # x-slice DMA caches (issued in need-order below)
            xv_tiles, xk_tiles, qx_tiles = {}, {}, {}

            def xv_tile(g, c):
                if (g, c) not in xv_tiles:
                    t = xvpool.tile([128, 1024], BF16, tag="xv")
                    nc.sync.dma_start(
                        t[:], xTv[c * 128:(c + 1) * 128,
                                  g * 1024:(g + 1) * 1024])
                    xv_tiles[(g, c)] = t
                return xv_tiles[(g, c)]

            def xk_tile(kq, c):
                if (kq, c) not in xk_tiles:
                    t = xkpool.tile([128, QC], BF16, tag="xk")
                    nc.sync.dma_start(
                        t[:], xTk[c * 128:(c + 1) * 128,
                                  kq * QC:(kq + 1) * QC])
                    xk_tiles[(kq, c)] = t
                return xk_tiles[(kq, c)]

            def qx_tile(q, c):
                if (q, c) not in qx_tiles:
                    t = qxpool.tile([128, QC], BF16, tag="qx")
                    nc.sync.dma_start(
                        t[:], xTq[c * 128:(c + 1) * 128,
                                  q * QC:(q + 1) * QC])
                    qx_tiles[(q, c)] = t
                return qx_tiles[(q, c)]

            # DMAs in need-order: S(0,0,*) inputs first, then V, rest of K
            nc.sync.dma_start(t_wk[:], wk[:])
            for c in range(NDC):
                xk_tile(0, c)
            nc.sync.dma_start(t_wq[:], wq[:])
            for c in range(NDC):
                qx_tile(0, c)
            nc.sync.dma_start(t_wv[:], wv[:])
            nc.sync.dma_start(t_vm[:], vmask[:])
            for c in range(NDC):
                xv_tile(0, c)
            for c in range(NDC):
                xk_tile(1, c)
            if maxtr > 8:
                for c in range(NDC):
                    xv_tile(1, c)
            for kq in (2, 3):
                for c in range(NDC):
                    xk_tile(kq, c)
            nc.sync.dma_start(t_wo[0][:], wo[0:128, :])
            nc.sync.dma_start(t_wo[1][:], wo[128:256, :])

            # persistent SBUF tensors
            t_qT = [qkpool.tile([128, N], BF16, tag=f"qT{p}", name=f"t_qT{p}")
                    for p in range(2)]
            t_kT = [qkpool.tile([128, N], BF16, tag=f"kT{p}", name=f"t_kT{p}")
                    for p in range(2)]
            t_v1 = [v1pool.tile([128, 65 * trips[j]], BF16, tag=f"v1_{j}",
                                name=f"t_v1_{j}")
                    for j in range(HPC)]
            t_pb = [pbpool.tile([128, N], BF16, tag=f"pb{p}", name=f"t_pb{p}")
                    for p in range(2)]

            # ---- unit builders ----
            def vproj_unit(t):
                """One V k-tile: 8 accumulating matmuls + masked copies."""
                def u():
                    g, kt8 = divmod(t, 8)
                    acc = ap.tile([128, 256], F32, tag="mix", bufs=2,
                                  name=f"vacc{t}")
                    for c in range(NDC):
                        xt = xv_tile(g, c)
                        nc.tensor.matmul(
                            acc[:], xt[:, kt8 * 128:(kt8 + 1) * 128],
                            t_wv[:, c * 256:(c + 1) * 256],
                            start=(c == 0), stop=(c == NDC - 1))
                    for j in range(HPC):
                        if t >= trips[j]:
                            continue
                        mask_col = t_vm[:, j * NKT + t: j * NKT + t + 1]
                        with nc.allow_low_precision(reason="bf16 V"):
                            nc.vector.tensor_scalar(
                                t_v1[j][:, t * 65: t * 65 + 64],
                                acc[:, j * 64:(j + 1) * 64],
                                mask_col, None, MUL)
                            nc.vector.tensor_copy(
                                t_v1[j][:, t * 65 + 64: t * 65 + 65],
                                mask_col)
                return u

            def kproj_unit(kq, m):
                """K projection m-half of one 512-col chunk."""
                def u():
                    acc = ap.tile([128, QC], F32, tag="mix", bufs=2,
                                  name=f"kacc{kq}_{m}")
                    for c in range(NDC):
                        nc.tensor.matmul(
                            acc[:],
                            t_wk[:, c * 256 + m * 128: c * 256 + (m + 1) * 128],
                            xk_tile(kq, c)[:],
                            start=(c == 0), stop=(c == NDC - 1))
                    with nc.allow_low_precision(reason="bf16 kT"):
                        nc.vector.tensor_copy(
                            t_kT[m][:, kq * QC:(kq + 1) * QC], acc[:])
                return u

            def qproj_unit(q, m):
                """Q projection m-half of chunk q."""
                def u():
                    acc = ap.tile([128, QC], F32, tag="mix", bufs=2,
                                  name=f"qacc{q}_{m}")
                    for c in range(NDC):
                        nc.tensor.matmul(
                            acc[:],
                            t_wq[:, c * 256 + m * 128: c * 256 + (m + 1) * 128],
                            qx_tile(q, c)[:],
                            start=(c == 0), stop=(c == NDC - 1))
                    with nc.allow_low_precision(reason="bf16 qT"):
                        nc.vector.tensor_copy(
                            t_qT[m][:, q * QC:(q + 1) * QC], acc[:])
                return u

            def outproj_units(q, alt_tags=False):
                """8 units: (qt, ch) matmul-pair + stage copy + dma.

                alt_tags (final batch): alternate PSUM tags and stage-copy
                engines (ACT is idle during the tail) for a deeper pipeline.
                """
                units = []

                def mk(qt, ch, tag, use_act):
                    def u():
                        ts = slice(qt * 128, (qt + 1) * 128)
                        o_ps = ap.tile([128, QC], F32, tag=tag, bufs=2,
                                       name=f"ops_{qt}_{ch}")
                        nc.tensor.matmul(
                            o_ps[:], t_pb[0][:, ts],
                            t_wo[0][:, ch * QC:(ch + 1) * QC],
                            start=True, stop=False)
                        nc.tensor.matmul(
                            o_ps[:], t_pb[1][:, ts],
                            t_wo[1][:, ch * QC:(ch + 1) * QC],
                            start=False, stop=True)
                        stg = opool.tile([128, QC], F32, tag="ostg")
                        if use_act:
                            nc.scalar.activation(stg[:], o_ps[:], AF.Copy)
                        else:
                            nc.vector.tensor_copy(stg[:], o_ps[:])
                        nc.sync.dma_start(out[ts, ch * QC:(ch + 1) * QC],
                                          stg[:])
                    return u

                i = 0
                for qt in range(q * (QC // 128), (q + 1) * (QC // 128)):
                    for ch in range(2):
                        tag = ("mix", "acc2")[i % 2] if alt_tags else "mix"
                        units.append(mk(qt, ch, tag, alt_tags and i % 2 == 1))
                        i += 1
                return units

            def normalize_e(q, p, e, acc):
                qs = slice(q * QC, (q + 1) * QC)
                if DEBUG_DUMP and q == 0 and p == 0:
                    stg = opool.tile([65, QC], F32, tag="dbga")
                    nc.vector.tensor_copy(stg[:], acc[:])
                    nc.sync.dma_start(
                        dbg["acc_a00" if e == 0 else "acc_b00"][:], stg[:])
                # reciprocal_approx_fast misreads PSUM inputs on HW;
                # stage the denominator row through SBUF first
                den = nrmpool.tile([1, QC], F32, tag="den")
                nc.vector.tensor_copy(den[:], acc[64:65, :])
                r = nrmpool.tile([1, QC], F32, tag="r")
                nc.vector.reciprocal_approx_fast(r[:], den[:])
                if DEBUG_DUMP and q == 0 and p == 0:
                    nc.sync.dma_start(dbg["r00"][e:e + 1, :], r[:])
                bc = nrmpool.tile([64, QC], F32, tag="bc")
                nc.gpsimd.partition_broadcast(bc[:], r[:])
                with nc.allow_low_precision(reason="bf16 heads"):
                    nc.vector.tensor_mul(
                        t_pb[p][e * 64:(e + 1) * 64, qs],
                        acc[0:64, :], bc[:])

            def ladder(q, p, fillers, delay=0):
                """S/exp/PV ladder for chunk q, slot pair p.

                Per t: S_a, S_b (concurrent PE row-groups), fused exp,
                PV_a, PV_b. `fillers` is a list of (fn, deadline) pairs;
                a unit with deadline d MUST be emitted before iteration d's
                ladder ops (it is emitted at the top of iteration d-2 at
                the latest); deadline None spreads evenly from `delay`.
                Slot b normalizes when its accumulator stops (hidden under
                the slot-a tail); slot a right after the loop.
                """
                qs = slice(q * QC, (q + 1) * QC)
                ja, jb = 2 * p, 2 * p + 1
                A, Bt = trips[ja], trips[jb]
                acc_b = ap.tile([65, QC], F32, tag="acc2", bufs=2,
                                name=f"acc_b{q}{p}")
                acc_a = ap.tile([65, QC], F32, tag="acc2", bufs=2,
                                name=f"acc_a{q}{p}")
                pending = list(fillers)
                n_total = len(pending)
                n_done = 0
                for t in range(A):
                    # deadline fillers first (1-iteration lookahead)
                    rest = []
                    for fn, dl in pending:
                        if dl is not None and dl <= t + 1:
                            fn()
                            n_done += 1
                        else:
                            rest.append((fn, dl))
                    pending = rest
                    both = t < Bt
                    ks = slice(t * 128, (t + 1) * 128)
                    sT = ap.tile([128, 2 * QC], F32, tag="sT", bufs=2,
                                 name="sT")
                    nc.tensor.matmul(sT[:, 0:QC], t_kT[p][0:64, ks],
                                     t_qT[p][0:64, qs],
                                     start=True, stop=True)
                    if both:
                        nc.tensor.matmul(sT[:, QC:2 * QC],
                                         t_kT[p][64:128, ks],
                                         t_qT[p][64:128, qs],
                                         start=True, stop=True)
                    w = 2 * QC if both else QC
                    pT = ptpool.tile([128, 2 * QC], BF16, tag="pT")
                    with nc.allow_low_precision(reason="bf16 probs"):
                        nc.scalar.activation(pT[:, 0:w], sT[:, 0:w],
                                             AF.Exp, scale=0.125)
                    if DEBUG_DUMP and q == 0 and p == 0 and t == 0:
                        stg = opool.tile([128, 2 * QC], F32, tag="dbgs")
                        nc.vector.tensor_copy(stg[:], sT[:])
                        nc.sync.dma_start(dbg["sT00"][:], stg[:])
                        nc.sync.dma_start(dbg["pT00"][:], pT[:])
                    nc.tensor.matmul(
                        acc_a[:], t_v1[ja][:, t * 65:(t + 1) * 65],
                        pT[:, 0:QC], start=(t == 0), stop=(t == A - 1))
                    if both:
                        nc.tensor.matmul(
                            acc_b[:], t_v1[jb][:, t * 65:(t + 1) * 65],
                            pT[:, QC:2 * QC], start=(t == 0),
                            stop=(t == Bt - 1))
                    if t == Bt - 1 and Bt < A:
                        normalize_e(q, p, 1, acc_b)
                    if t >= delay:
                        want = ((t + 1 - delay) * n_total
                                // max(A - delay, 1))
                        while pending and n_done < want:
                            fn, _ = pending.pop(0)
                            fn()
                            n_done += 1
                if Bt == A:
                    normalize_e(q, p, 1, acc_b)
                normalize_e(q, p, 0, acc_a)
                for fn, _ in pending:
                    fn()

            # ---- serial head: only what the first S matmul needs ----
            kproj_unit(0, 0)()
            qproj_unit(0, 0)()

            # everything else rides ladder(0,0) as deadline fillers:
            # V tile t feeds PV at iteration t; K chunk kq's m0 feeds S at
            # iteration 4*kq; the m1 halves (pair 1) and q0 m1 are only
            # needed by ladder(0,1) and spread freely.
            f00 = [(vproj_unit(t), t) for t in range(maxtr)]
            for kq in (1, 2, 3):
                f00.insert(4 * kq, (kproj_unit(kq, 0), 4 * kq))
            f00.extend([(kproj_unit(0, 1), None),
                        (qproj_unit(0, 1), None),
                        (kproj_unit(1, 1), None),
                        (kproj_unit(2, 1), None),
                        (kproj_unit(3, 1), None)])

            for q in range(NCH):
                if q == 0:
                    ladder(q, 0, f00)
                else:
                    ladder(q, 0, [(u, None) for u in outproj_units(q - 1)],
                           delay=3)
                f1 = ([(qproj_unit(q + 1, 0), None),
                       (qproj_unit(q + 1, 1), None)]
                      if q < NCH - 1 else [])
                if q < NCH - 1:
                    for c in range(NDC):
                        qx_tile(q + 1, c)  # prefetch
                ladder(q, 1, f1)
            for u in outproj_units(NCH - 1, alt_tags=True):
                u()

            if DEBUG_DUMP:
                for p in range(2):
                    nc.sync.dma_start(dbg[f"qT{p}"][:], t_qT[p][:])
                    nc.sync.dma_start(dbg[f"kT{p}"][:], t_kT[p][:])
                    nc.sync.dma_start(dbg[f"pb{p}"][:], t_pb[p][:])
                for j in range(HPC):
                    nc.sync.dma_start(dbg[f"v1_{j}"][:], t_v1[j][:])

    nc.finalize()
    return nc


def kernel(queries, keys, values, valid_len, Wq, Wk, Wv, Wo):
    global LAST_RESULTS
    queries = np.asarray(queries, dtype=np.float32)
    keys = np.asarray(keys, dtype=np.float32)
    values = np.asarray(values, dtype=np.float32)
    Wq = np.asarray(Wq, dtype=np.float32)
    Wk = np.asarray(Wk, dtype=np.float32)
    Wv = np.asarray(Wv, dtype=np.float32)
    Wo = np.asarray(Wo, dtype=np.float32)
    vl = np.asarray(valid_len).astype(np.int64).reshape(B * H)

    # rank-aligned slot assignment: per batch, heads sorted by vl desc;
    # slot j of the 4 cores of that batch takes ranks 4j..4j+3
    order = {}
    for b in range(B):
        idx = (np.argsort(-vl[b * H:(b + 1) * H], kind="stable") + b * H)
        for cg in range(4):
            order[b * 4 + cg] = [int(idx[4 * j + cg]) for j in range(HPC)]
    trips = []
    for j in range(HPC):
        m = max(int(-(-vl[order[c][j]] // 128)) for c in range(NCORES))
        trips.append(max(1, min(NKT, m)))

    nc = _build_program(tuple(trips))

    in_maps = []
    for c in range(NCORES):
        b = c // 4
        heads = order[c]
        cols = np.concatenate(
            [np.arange((h - b * H) * DH, (h - b * H + 1) * DH) for h in heads])

        def wlayout(w):
            return np.ascontiguousarray(
                w[:, cols].reshape(NDC, 128, 256).transpose(1, 0, 2)
                .reshape(128, NDC * 256)).astype(BFNP)

        vm = np.zeros((128, HPC * NKT), np.float32)
        for j, h in enumerate(heads):
            keep = (np.arange(N) < vl[h]).astype(np.float32)
            vm[:, j * NKT:(j + 1) * NKT] = keep.reshape(NKT, 128).T

        in_maps.append({
            "xTq": np.ascontiguousarray(queries[b].T).astype(BFNP),
            "xTk": np.ascontiguousarray(keys[b].T).astype(BFNP),
            "xTv": np.ascontiguousarray(values[b].T).astype(BFNP),
            "wq": wlayout(Wq),
            "wk": wlayout(Wk),
            "wv": wlayout(Wv),
            "wo": np.ascontiguousarray(Wo[cols, :]).astype(BFNP),
            "vmask": vm,
        })

    LAST_RESULTS = run_bass_kernel_spmd(nc, in_maps, list(range(NCORES)))
    res = LAST_RESULTS.results

    out = np.zeros((B, N, D), np.float64)
    for c in range(NCORES):
        out[c // 4] += res[c]["out"].astype(np.float64)
    return out.astype(np.float32)


# revision 26
# speedup vs baseline: 1.0191x; 1.0191x over previous
"""Multi-head attention TRN2 kernel (8 NeuronCores, SPMD).

Problem: B=2, N=2048, D=1024, H=16 heads of dim 64, fp32 in/out, per-(b,h)
key-length masking (valid_len, length 32).

Sharding: batch*heads across 8 cores - core c handles batch b=c//4 and 4
heads ("slots"), rank-aligned by valid_len so the per-slot key-tile trip
counts (uniform across cores, specialized at build time) are minimal.

v4 design (HAM-warm, ACT-paced, minimal serial head):
  - All matmul operands bf16 (host converts; PSUM accumulation stays fp32).
  - Junk matmuls at t=0 warm the HAM clock gate while weights DMA.
  - Serial head is only: V proj k-tiles 0..7, K proj chunk 0, Q proj chunk
    0. The rest of K proj and V proj ride as fillers inside the first
    attention ladder, using the spare "mix" PSUM banks; Q proj of chunk
    q+1 and out-proj of chunk q-1 ride inside later ladders.
  - S^T per slot-pair: two K=64 matmuls emitted back-to-back run
    concurrently in different PE row-groups (tile_position auto-derived).
    Both halves land in one [128,1024] 2-bank PSUM tile; ONE fused Exp
    (scale=1/8) covers both slots -> bf16 pT.
  - Softmax denominator via masked-ones column in V1.
  - Normalize: DVE reciprocal_approx_fast (SBUF-staged; the custom op
    misreads PSUM on HW) + GpSimd partition_broadcast + DVE multiply.
    Slot b normalizes as soon as its accumulator stops (hidden in ladder).
  - PSUM budget (8 banks): sT 2x2 + acc2 2 + mix 2.
"""
import sys
import numpy as np
from contextlib import ExitStack

sys.path.insert(0, "/opt/trn_rl_repo")

import ml_dtypes  # noqa: E402
import concourse.bass as bass  # noqa: E402
from concourse import bacc, mybir  # noqa: E402
import concourse.tile as tile  # noqa: E402
from concourse.bass_utils import run_bass_kernel_spmd  # noqa: E402

F32 = mybir.dt.float32
BF16 = mybir.dt.bfloat16
AF = mybir.ActivationFunctionType
MUL = mybir.AluOpType.mult
BFNP = ml_dtypes.bfloat16

B, N, D, H = 2, 2048, 1024, 16
DH = 64
HPC = 4          # heads (slots) per core
NCORES = 8
QC = 512         # q chunk (matmul free dim)
NKT = N // 128   # 16 k tiles
NDC = D // 128   # 8 contraction chunks
NCH = N // QC    # 4 q chunks

LAST_RESULTS = None  # BassKernelResults of the most recent run (for tooling)
DEBUG_DUMP = False   # add DRAM dumps of intermediates (debugging only)


def _build_program(trips):
    """trips: 4 ints - k-tile count per slot (uniform across cores)."""
    nc = bacc.Bacc("TRN2", target_bir_lowering=False, debug=False,
                   num_devices=NCORES)
    maxtr = max(trips)

    xTq = nc.dram_tensor("xTq", [D, N], BF16, kind="ExternalInput")
    xTk = nc.dram_tensor("xTk", [D, N], BF16, kind="ExternalInput")
    xTv = nc.dram_tensor("xTv", [D, N], BF16, kind="ExternalInput")
    wq = nc.dram_tensor("wq", [128, NDC * 256], BF16, kind="ExternalInput")
    wk = nc.dram_tensor("wk", [128, NDC * 256], BF16, kind="ExternalInput")
    wv = nc.dram_tensor("wv", [128, NDC * 256], BF16, kind="ExternalInput")
    wo = nc.dram_tensor("wo", [256, D], BF16, kind="ExternalInput")
    vmask = nc.dram_tensor("vmask", [128, HPC * NKT], F32, kind="ExternalInput")
    out = nc.dram_tensor("out", [N, D], F32, kind="ExternalOutput")
    dbg = {}
    if DEBUG_DUMP:
        for p in range(2):
            dbg[f"qT{p}"] = nc.dram_tensor(f"d_qT{p}", [128, N], BF16,
                                           kind="ExternalOutput")
            dbg[f"kT{p}"] = nc.dram_tensor(f"d_kT{p}", [128, N], BF16,
                                           kind="ExternalOutput")
            dbg[f"pb{p}"] = nc.dram_tensor(f"d_pb{p}", [128, N], BF16,
                                           kind="ExternalOutput")
        for j in range(HPC):
            dbg[f"v1_{j}"] = nc.dram_tensor(f"d_v1_{j}", [128, 65 * trips[j]],
                                            BF16, kind="ExternalOutput")
        dbg["sT00"] = nc.dram_tensor("d_sT00", [128, 1024], F32,
                                     kind="ExternalOutput")
        dbg["pT00"] = nc.dram_tensor("d_pT00", [128, 1024], BF16,
                                     kind="ExternalOutput")
        dbg["acc_a00"] = nc.dram_tensor("d_acc_a00", [65, QC], F32,
                                        kind="ExternalOutput")
        dbg["acc_b00"] = nc.dram_tensor("d_acc_b00", [65, QC], F32,
                                        kind="ExternalOutput")
        dbg["r00"] = nc.dram_tensor("d_r00", [2, QC], F32,
                                    kind="ExternalOutput")

    with tile.TileContext(nc) as tc:
        with ExitStack() as ctx:
            wpool = ctx.enter_context(tc.tile_pool(name="wpool", bufs=1))
            xvpool = ctx.enter_context(tc.tile_pool(name="xvpool", bufs=16))
            xkpool = ctx.enter_context(tc.tile_pool(name="xkpool", bufs=32))
            qxpool = ctx.enter_context(tc.tile_pool(name="qxpool", bufs=16))
            qkpool = ctx.enter_context(tc.tile_pool(name="qkpool", bufs=1))
            v1pool = ctx.enter_context(tc.tile_pool(name="v1pool", bufs=1))
            ptpool = ctx.enter_context(tc.tile_pool(name="ptpool", bufs=3))
            nrmpool = ctx.enter_context(tc.tile_pool(name="nrmpool", bufs=2))
            pbpool = ctx.enter_context(tc.tile_pool(name="pbpool", bufs=1))
            opool = ctx.enter_context(tc.tile_pool(name="opool", bufs=3))
            ap = ctx.enter_context(tc.tile_pool(name="ap", bufs=1,
                                                space="PSUM"))

            # ---- HAM warm-up: junk matmuls while weights stream in ----
            t_junk = wpool.tile([128, QC], BF16, tag="junk")
            nc.vector.memset(t_junk[:], 0.0)
            for i in range(10):
                wps = ap.tile([128, QC], F32, tag="mix", bufs=2,
                              name=f"warm{i}")
                nc.tensor.matmul(wps[:], t_junk[:, 0:128], t_junk[:],
                                 start=True, stop=True)

            t_wq = wpool.tile([128, NDC * 256], BF16, tag="wq")
            t_wk = wpool.tile([128, NDC * 256], BF16, tag="wk")
            t_wv = wpool.tile([128, NDC * 256], BF16, tag="wv")
            t_vm = wpool.tile([128, HPC * NKT], F32, tag="vm")
            t_wo = [wpool.tile([128, D], BF16, tag=f"wo{p}", name=f"t_wo{p}")
                    for p in range(2)]

            # warm the ACT exp table during the projection head
            t_scr = wpool.tile([1, 1], F32, tag="scr")
            nc.vector.memset(t_scr[:], 0.0)
            t_scr2 = wpool.tile([1, 1], F32, tag="scr2")
            nc.scalar.activation(t_scr2[:], t_scr[:], AF.Exp)

            # x-slice DMA caches (issued in need-order below)
            xv_tiles, xk_tiles, qx_tiles = {}, {}, {}

            def xv_tile(g, c):
                if (g, c) not in xv_tiles:
                    t = xvpool.tile([128, 1024], BF16, tag="xv")
                    nc.sync.dma_start(
                        t[:], xTv[c * 128:(c + 1) * 128,
                                  g * 1024:(g + 1) * 1024])
                    xv_tiles[(g, c)] = t
                return xv_tiles[(g, c)]

            def xk_tile(kq, c):
                if (kq, c) not in xk_tiles:
                    t = xkpool.tile([128, QC], BF16, tag="xk")
                    nc.sync.dma_start(
                        t[:], xTk[c * 128:(c + 1) * 128,
                                  kq * QC:(kq + 1) * QC])
                    xk_tiles[(kq, c)] = t
                return xk_tiles[(kq, c)]

            def qx_tile(q, c):
                if (q, c) not in qx_tiles:
                    t = qxpool.tile([128, QC], BF16, tag="qx")
                    nc.sync.dma_start(
                        t[:], xTq[c * 128:(c + 1) * 128,
                                  q * QC:(q + 1) * QC])
                    qx_tiles[(q, c)] = t
                return qx_tiles[(q, c)]

            # DMAs in need-order: all of K + Q chunk 0 first (phase A of the
            # decoupled chunk-0 ladder is S/exp only), then V, then wo
            nc.sync.dma_start(t_wk[:], wk[:])
            for c in range(NDC):
                xk_tile(0, c)
            nc.sync.dma_start(t_wq[:], wq[:])
            for c in range(NDC):
                qx_tile(0, c)
            for kq in (1, 2, 3):
                for c in range(NDC):
                    xk_tile(kq, c)
            nc.sync.dma_start(t_wv[:], wv[:])
            nc.sync.dma_start(t_vm[:], vmask[:])
            for c in range(NDC):
                xv_tile(0, c)
            if maxtr > 8:
                for c in range(NDC):
                    xv_tile(1, c)
            nc.sync.dma_start(t_wo[0][:], wo[0:128, :])
            nc.sync.dma_start(t_wo[1][:], wo[128:256, :])

            # persistent SBUF tensors
            t_qT = [qkpool.tile([128, N], BF16, tag=f"qT{p}", name=f"t_qT{p}")
                    for p in range(2)]
            t_kT = [qkpool.tile([128, N], BF16, tag=f"kT{p}", name=f"t_kT{p}")
                    for p in range(2)]
            t_v1 = [v1pool.tile([128, 65 * trips[j]], BF16, tag=f"v1_{j}",
                                name=f"t_v1_{j}")
                    for j in range(HPC)]
            t_pb = [pbpool.tile([128, N], BF16, tag=f"pb{p}", name=f"t_pb{p}")
                    for p in range(2)]

            # ---- unit builders ----
            def vproj_unit(t):
                """One V k-tile: 8 accumulating matmuls + masked copies."""
                def u():
                    g, kt8 = divmod(t, 8)
                    acc = ap.tile([128, 256], F32, tag="mix", bufs=2,
                                  name=f"vacc{t}")
                    for c in range(NDC):
                        xt = xv_tile(g, c)
                        nc.tensor.matmul(
                            acc[:], xt[:, kt8 * 128:(kt8 + 1) * 128],
                            t_wv[:, c * 256:(c + 1) * 256],
                            start=(c == 0), stop=(c == NDC - 1))
                    for j in range(HPC):
                        if t >= trips[j]:
                            continue
                        mask_col = t_vm[:, j * NKT + t: j * NKT + t + 1]
                        with nc.allow_low_precision(reason="bf16 V"):
                            nc.vector.tensor_scalar(
                                t_v1[j][:, t * 65: t * 65 + 64],
                                acc[:, j * 64:(j + 1) * 64],
                                mask_col, None, MUL)
                            nc.vector.tensor_copy(
                                t_v1[j][:, t * 65 + 64: t * 65 + 65],
                                mask_col)
                return u

            def kproj_unit(kq, m):
                """K projection m-half of one 512-col chunk."""
                def u():
                    acc = ap.tile([128, QC], F32, tag="mix", bufs=2,
                                  name=f"kacc{kq}_{m}")
                    for c in range(NDC):
                        nc.tensor.matmul(
                            acc[:],
                            t_wk[:, c * 256 + m * 128: c * 256 + (m + 1) * 128],
                            xk_tile(kq, c)[:],
                            start=(c == 0), stop=(c == NDC - 1))
                    with nc.allow_low_precision(reason="bf16 kT"):
                        nc.vector.tensor_copy(
                            t_kT[m][:, kq * QC:(kq + 1) * QC], acc[:])
                return u

            def qproj_unit(q, m):
                """Q projection m-half of chunk q."""
                def u():
                    acc = ap.tile([128, QC], F32, tag="mix", bufs=2,
                                  name=f"qacc{q}_{m}")
                    for c in range(NDC):
                        nc.tensor.matmul(
                            acc[:],
                            t_wq[:, c * 256 + m * 128: c * 256 + (m + 1) * 128],
                            qx_tile(q, c)[:],
                            start=(c == 0), stop=(c == NDC - 1))
                    with nc.allow_low_precision(reason="bf16 qT"):
                        nc.vector.tensor_copy(
                            t_qT[m][:, q * QC:(q + 1) * QC], acc[:])
                return u

            def outproj_units(q, alt_tags=False):
                """8 units: (qt, ch) matmul-pair + stage copy + dma.

                alt_tags (final batch): alternate PSUM tags and stage-copy
                engines (ACT is idle during the tail) for a deeper pipeline.
                """
                units = []

                def mk(qt, ch, tag, use_act):
                    def u():
                        ts = slice(qt * 128, (qt + 1) * 128)
                        o_ps = ap.tile([128, QC], F32, tag=tag, bufs=2,
                                       name=f"ops_{qt}_{ch}")
                        nc.tensor.matmul(
                            o_ps[:], t_pb[0][:, ts],
                            t_wo[0][:, ch * QC:(ch + 1) * QC],
                            start=True, stop=False)
                        nc.tensor.matmul(
                            o_ps[:], t_pb[1][:, ts],
                            t_wo[1][:, ch * QC:(ch + 1) * QC],
                            start=False, stop=True)
                        stg = opool.tile([128, QC], F32, tag="ostg")
                        if use_act:
                            nc.scalar.activation(stg[:], o_ps[:], AF.Copy)
                        else:
                            nc.vector.tensor_copy(stg[:], o_ps[:])
                        nc.sync.dma_start(out[ts, ch * QC:(ch + 1) * QC],
                                          stg[:])
                    return u

                i = 0
                for qt in range(q * (QC // 128), (q + 1) * (QC // 128)):
                    for ch in range(2):
                        tag = ("mix", "acc2")[i % 2] if alt_tags else "mix"
                        units.append(mk(qt, ch, tag, alt_tags and i % 2 == 1))
                        i += 1
                return units

            def normalize_e(q, p, e, acc):
                qs = slice(q * QC, (q + 1) * QC)
                if DEBUG_DUMP and q == 0 and p == 0:
                    stg = opool.tile([65, QC], F32, tag="dbga")
                    nc.vector.tensor_copy(stg[:], acc[:])
                    nc.sync.dma_start(
                        dbg["acc_a00" if e == 0 else "acc_b00"][:], stg[:])
                # reciprocal_approx_fast misreads PSUM inputs on HW;
                # stage the denominator row through SBUF first
                den = nrmpool.tile([1, QC], F32, tag="den")
                nc.vector.tensor_copy(den[:], acc[64:65, :])
                r = nrmpool.tile([1, QC], F32, tag="r")
                nc.vector.reciprocal_approx_fast(r[:], den[:])
                if DEBUG_DUMP and q == 0 and p == 0:
                    nc.sync.dma_start(dbg["r00"][e:e + 1, :], r[:])
                bc = nrmpool.tile([64, QC], F32, tag="bc")
                nc.gpsimd.partition_broadcast(bc[:], r[:])
                with nc.allow_low_precision(reason="bf16 heads"):
                    nc.vector.tensor_mul(
                        t_pb[p][e * 64:(e + 1) * 64, qs],
                        acc[0:64, :], bc[:])

            def ladder(q, p, fillers, delay=0):
                """S/exp/PV ladder for chunk q, slot pair p.

                Per t: S_a, S_b (concurrent PE row-groups), fused exp,
                PV_a, PV_b. `fillers` is a list of (fn, deadline) pairs;
                a unit with deadline d MUST be emitted before iteration d's
                ladder ops (it is emitted at the top of iteration d-2 at
                the latest); deadline None spreads evenly from `delay`.
                Slot b normalizes when its accumulator stops (hidden under
                the slot-a tail); slot a right after the loop.
                """
                qs = slice(q * QC, (q + 1) * QC)
                ja, jb = 2 * p, 2 * p + 1
                A, Bt = trips[ja], trips[jb]
                acc_b = ap.tile([65, QC], F32, tag="acc2", bufs=2,
                                name=f"acc_b{q}{p}")
                acc_a = ap.tile([65, QC], F32, tag="acc2", bufs=2,
                                name=f"acc_a{q}{p}")
                pending = list(fillers)
                n_total = len(pending)
                n_done = 0
                for t in range(A):
                    # deadline fillers first (1-iteration lookahead)
                    rest = []
                    for fn, dl in pending:
                        if dl is not None and dl <= t + 1:
                            fn()
                            n_done += 1
                        else:
                            rest.append((fn, dl))
                    pending = rest
                    both = t < Bt
                    ks = slice(t * 128, (t + 1) * 128)
                    sT = ap.tile([128, 2 * QC], F32, tag="sT", bufs=2,
                                 name="sT")
                    nc.tensor.matmul(sT[:, 0:QC], t_kT[p][0:64, ks],
                                     t_qT[p][0:64, qs],
                                     start=True, stop=True)
                    if both:
                        nc.tensor.matmul(sT[:, QC:2 * QC],
                                         t_kT[p][64:128, ks],
                                         t_qT[p][64:128, qs],
                                         start=True, stop=True)
                    w = 2 * QC if both else QC
                    pT = ptpool.tile([128, 2 * QC], BF16, tag="pT")
                    with nc.allow_low_precision(reason="bf16 probs"):
                        nc.scalar.activation(pT[:, 0:w], sT[:, 0:w],
                                             AF.Exp, scale=0.125)
                    if DEBUG_DUMP and q == 0 and p == 0 and t == 0:
                        stg = opool.tile([128, 2 * QC], F32, tag="dbgs")
                        nc.vector.tensor_copy(stg[:], sT[:])
                        nc.sync.dma_start(dbg["sT00"][:], stg[:])
                        nc.sync.dma_start(dbg["pT00"][:], pT[:])
                    nc.tensor.matmul(
                        acc_a[:], t_v1[ja][:, t * 65:(t + 1) * 65],
                        pT[:, 0:QC], start=(t == 0), stop=(t == A - 1))
                    if both:
                        nc.tensor.matmul(
                            acc_b[:], t_v1[jb][:, t * 65:(t + 1) * 65],
                            pT[:, QC:2 * QC], start=(t == 0),
                            stop=(t == Bt - 1))
                    if t == Bt - 1 and Bt < A:
                        normalize_e(q, p, 1, acc_b)
                    if t >= delay:
                        want = ((t + 1 - delay) * n_total
                                // max(A - delay, 1))
                        while pending and n_done < want:
                            fn, _ = pending.pop(0)
                            fn()
                            n_done += 1
                if Bt == A:
                    normalize_e(q, p, 1, acc_b)
                normalize_e(q, p, 0, acc_a)
                for fn, _ in pending:
                    fn()

            def ladder0_split(fillers):
                """Chunk 0, pair 0, decoupled: phase A runs S+exp only
                (needs just K/Q data, which DMAs first), buffering all exp
                outputs; phase B streams V-proj units interleaved with the
                deferred PV matmuls once xTv has landed. This keeps ACT
                busy from ~10us while the 12MB input load is still in
                flight."""
                qs = slice(0, QC)
                A, Bt = trips[0], trips[1]
                pending = list(fillers)
                n_total = len(pending)
                n_done = 0
                pts = []
                for t in range(A):
                    rest = []
                    for fn, dl in pending:
                        if dl is not None and dl <= t + 1:
                            fn()
                            n_done += 1
                        else:
                            rest.append((fn, dl))
                    pending = rest
                    both = t < Bt
                    ks = slice(t * 128, (t + 1) * 128)
                    sT = ap.tile([128, 2 * QC], F32, tag="sT", bufs=2,
                                 name="sT")
                    nc.tensor.matmul(sT[:, 0:QC], t_kT[0][0:64, ks],
                                     t_qT[0][0:64, qs],
                                     start=True, stop=True)
                    if both:
                        nc.tensor.matmul(sT[:, QC:2 * QC],
                                         t_kT[0][64:128, ks],
                                         t_qT[0][64:128, qs],
                                         start=True, stop=True)
                    w = 2 * QC if both else QC
                    pT = ptpool.tile([128, 2 * QC], BF16, tag="pT0",
                                     bufs=max(A, 1))
                    with nc.allow_low_precision(reason="bf16 probs"):
                        nc.scalar.activation(pT[:, 0:w], sT[:, 0:w],
                                             AF.Exp, scale=0.125)
                    pts.append((pT, both))
                    want = (t + 1) * n_total // A
                    while pending and n_done < want:
                        fn, _ = pending.pop(0)
                        fn()
                        n_done += 1
                for fn, _ in pending:
                    fn()
                # phase B: V units just-in-time ahead of their PVs
                acc_b = ap.tile([65, QC], F32, tag="acc2", bufs=2,
                                name="acc_b00")
                acc_a = ap.tile([65, QC], F32, tag="acc2", bufs=2,
                                name="acc_a00")
                vproj_unit(0)()
                for t in range(A):
                    if t + 1 < maxtr:
                        vproj_unit(t + 1)()
                    pT, both = pts[t]
                    nc.tensor.matmul(
                        acc_a[:], t_v1[0][:, t * 65:(t + 1) * 65],
                        pT[:, 0:QC], start=(t == 0), stop=(t == A - 1))
                    if both:
                        nc.tensor.matmul(
                            acc_b[:], t_v1[1][:, t * 65:(t + 1) * 65],
                            pT[:, QC:2 * QC], start=(t == 0),
                            stop=(t == Bt - 1))
                    if t == Bt - 1 and Bt < A:
                        normalize_e(0, 0, 1, acc_b)
                if Bt == A:
                    normalize_e(0, 0, 1, acc_b)
                normalize_e(0, 0, 0, acc_a)

            # ---- serial head: only what the first S matmul needs ----
            kproj_unit(0, 0)()
            qproj_unit(0, 0)()

            # phase-A fillers: K chunk kq's m0 feeds S at iteration 4*kq;
            # the m1 halves and q0 m1 are only needed by ladder(0,1).
            f00 = [(kproj_unit(kq, 0), 4 * kq) for kq in (1, 2, 3)]
            f00.extend([(kproj_unit(0, 1), None),
                        (qproj_unit(0, 1), None),
                        (kproj_unit(1, 1), None),
                        (kproj_unit(2, 1), None),
                        (kproj_unit(3, 1), None)])

            for q in range(NCH):
                if q == 0:
                    ladder0_split(f00)
                else:
                    ladder(q, 0, [(u, None) for u in outproj_units(q - 1)],
                           delay=3)
                f1 = ([(qproj_unit(q + 1, 0), None),
                       (qproj_unit(q + 1, 1), None)]
                      if q < NCH - 1 else [])
                if q < NCH - 1:
                    for c in range(NDC):
                        qx_tile(q + 1, c)  # prefetch
                ladder(q, 1, f1)
            for u in outproj_units(NCH - 1, alt_tags=True):
                u()

            if DEBUG_DUMP:
                for p in range(2):
                    nc.sync.dma_start(dbg[f"qT{p}"][:], t_qT[p][:])
                    nc.sync.dma_start(dbg[f"kT{p}"][:], t_kT[p][:])
                    nc.sync.dma_start(dbg[f"pb{p}"][:], t_pb[p][:])
                for j in range(HPC):
                    nc.sync.dma_start(dbg[f"v1_{j}"][:], t_v1[j][:])

    nc.finalize()
    return nc


def kernel(queries, keys, values, valid_len, Wq, Wk, Wv, Wo):
    global LAST_RESULTS
    queries = np.asarray(queries, dtype=np.float32)
    keys = np.asarray(keys, dtype=np.float32)
    values = np.asarray(values, dtype=np.float32)
    Wq = np.asarray(Wq, dtype=np.float32)
    Wk = np.asarray(Wk, dtype=np.float32)
    Wv = np.asarray(Wv, dtype=np.float32)
    Wo = np.asarray(Wo, dtype=np.float32)
    vl = np.asarray(valid_len).astype(np.int64).reshape(B * H)

    # rank-aligned slot assignment: per batch, heads sorted by vl desc;
    # slot j of the 4 cores of that batch takes ranks 4j..4j+3
    order = {}
    for b in range(B):
        idx = (np.argsort(-vl[b * H:(b + 1) * H], kind="stable") + b * H)
        for cg in range(4):
            order[b * 4 + cg] = [int(idx[4 * j + cg]) for j in range(HPC)]
    trips = []
    for j in range(HPC):
        m = max(int(-(-vl[order[c][j]] // 128)) for c in range(NCORES))
        trips.append(max(1, min(NKT, m)))

    nc = _build_program(tuple(trips))

    in_maps = []
    for c in range(NCORES):
        b = c // 4
        heads = order[c]
        cols = np.concatenate(
            [np.arange((h - b * H) * DH, (h - b * H + 1) * DH) for h in heads])

        def wlayout(w):
            return np.ascontiguousarray(
                w[:, cols].reshape(NDC, 128, 256).transpose(1, 0, 2)
                .reshape(128, NDC * 256)).astype(BFNP)

        vm = np.zeros((128, HPC * NKT), np.float32)
        for j, h in enumerate(heads):
            keep = (np.arange(N) < vl[h]).astype(np.float32)
            vm[:, j * NKT:(j + 1) * NKT] = keep.reshape(NKT, 128).T

        in_maps.append({
            "xTq": np.ascontiguousarray(queries[b].T).astype(BFNP),
            "xTk": np.ascontiguousarray(keys[b].T).astype(BFNP),
            "xTv": np.ascontiguousarray(values[b].T).astype(BFNP),
            "wq": wlayout(Wq),
            "wk": wlayout(Wk),
            "wv": wlayout(Wv),
            "wo": np.ascontiguousarray(Wo[cols, :]).astype(BFNP),
            "vmask": vm,
        })

    LAST_RESULTS = run_bass_kernel_spmd(nc, in_maps, list(range(NCORES)))
    res = LAST_RESULTS.results

    out = np.zeros((B, N, D), np.float64)
    for c in range(NCORES):
        out[c // 4] += res[c]["out"].astype(np.float64)
    return out.astype(np.float32)
